# revision 1
# baseline (speedup 1.0000x reference)
"""GNN message passing (GraphConv x3 + TopKPooling + MLP head) on 8 trn2 cores.

Strategy: shard the 128 graphs across 8 cores (16 graphs/core). Each graph's
dense 512x512 adjacency-count matrix AT[s, d] = #edges s->d is built ON
DEVICE from the raw edge list (shipping AT from host was 8.4MB/core and the
axon-tunnel upload dominated wall time; the edge list is 0.6MB/core).
Edges are host-sorted by src block (s>>7) and padded to CPB 128-edge chunks
per block; per chunk, DVE compares an f16 iota row against the src/dst
index columns (is_equal with f32 per-partition scalar -> exact {0,1} f16
one-hots S [128x128], D [128x512]) and PE accumulates AT_b += S^T @ D into
one PSUM bank per block — exact integer counts in f32, exact in f16.
All f16 inputs ride in ONE flat dram parameter and all f32 in another
(per-launch dispatch cost over the axon tunnel scales with arg count).
Message passing then proceeds as dense PE matmuls (fp16 operands, f32 PSUM
accumulate):

  A-step:  U_l = (A @ h_l)^T : stat = h_l (normal [node, feat]), mov = AT
           -> PSUM [feat, node] ("T-land")
  W-step(a): H_{l+1} = relu(Wst_l^T @ [H_l; U_l] + b_l)    (T-land)
  W-step(b): h_{l+1} = relu([H_l; U_l]^T-stat @ Wst_l-mov) (normal)
           -- same product with swapped stationary/moving roles; this yields
              both layouts without any transpose instruction.

Pools: mean pools ride the Act-engine relu drains (accum_out); max pools are
DVE free-dim reduces. Top-k keeps the 410 best scores per graph: the 103rd
smallest score is found with 13 rounds of DVE max8/match_replace on negated
scores, then masked pooling (mask/tanh rows broadcast to 128 partitions via
a DRAM-replicated DMA; elementwise on GPSIMD, reduces on DVE).

Hardware quirks handled: SBUF partition bases must be in {0,32,64,96} for
every instruction, so per-graph score rows are accumulated into a [4, 512]
batch tile via one-cold-column stationary matmuls; the walrus build also
rejects dma_transpose / gpsimd-extended ops / tensor_tensor_reduce, none of
which are used.
"""

import contextlib
import re

import numpy as np

NCORES = 8
G = 16          # graphs per core
N = 512         # nodes per graph
D = 256         # embed dim
E = 8192        # edges per graph
K = 410         # top-k kept per graph (ceil(0.8 * 512))
CPB = 18        # padded 128-edge chunks per src-block (max needed here: 17)
BATCHES = [(0, 6), (6, 6), (12, 4)]  # sized so each topk+x3 overlaps remaining PE work
TBMAX = 7

F16 = np.float16
# h3 >= 0 (post-relu) and the top-410 always contains a node with tanh >= 0,
# so masked-max == max(H3*w) with excluded entries zeroed by the mask.


def layout16(cpb):
    """Flat f16 input-blob layout (order shared by build_program/pack_inputs).
    All f16 inputs ride in ONE dram parameter: per-launch dispatch cost over
    the axon tunnel scales with argument count (~120us/arg), so 21 params
    -> 3 (f16 blob, f32 blob, output)."""
    return [
        ("sd", G * 128 * 8 * cpb),
        ("iota", N),
        ("xs", G * N * 4),
        ("xt", G * 4 * N),
        ("ws1", 4 * D), ("wr1", 4 * D),
        ("wst2", 2 * D * D), ("wst3", 2 * D * D),
        ("b1r", D), ("b2r", D),
        ("pmat", 128 * 2 * G * TBMAX),
        ("mw1", 2 * D * D), ("mw2", D * (D // 2)), ("mw3", D // 2),
    ]


LAYOUT32 = [
    ("prow", D),
    ("b1", D), ("b2", D), ("b3", D),
    ("c1", D), ("c2", D // 2), ("c3", 1),
]


def _offsets(layout):
    offs, pos = {}, 0
    for name, n in layout:
        offs[name] = (pos, n)
        pos += n
    return offs, pos


_PATCHED = False


def _apply_tile_patch():
    """walrus rejects >1 sem-wait on the final SP Drain: split into nops."""
    global _PATCHED
    if _PATCHED:
        return
    import bass_rust
    from concourse.tile import TileContext
    from concourse.vector_clock import ScopedClock

    def _patched(self, tick_clock, wait_clock):
        vals = [int(x) for x in re.findall(r"\d+", repr(tick_clock.global_clock))]
        for i, v in enumerate(vals):
            if v <= 0:
                continue
            single = [0] * len(vals)
            single[i] = v
            nop_inst = self.nc.sync.nop(nofuse=True, hint=f"split_drain_{i}")
            wait_clock.add_sem_waits(
                nop_inst.ins, ScopedClock({None: bass_rust.VectorClock(single)})
            )
        self.nc.sync.drain()
        self.nc.all_engine_barrier()
        assert self.sems is not None
        popped = self.nc._tile_sem_poison_stack.pop()
        assert popped is self._sem_poison
        self.nc.clear_and_free_semaphores(list(self.sems.allocated().values()))
        self.nc.all_engine_barrier()

    TileContext._drain_and_barrier = _patched
    _PATCHED = True


def _split_multi_waits(nc):
    """walrus allows only one sem-wait per instruction: hoist extras onto
    injected same-engine nops placed immediately before the instruction
    (per-engine program order makes the earlier wait a safe strengthening)."""
    import bass_rust
    import concourse.mybir as mybir

    n = 0
    for fn in nc.m.functions:
        for bb in fn.blocks:
            out = []
            for inst in bb.instructions:
                si = inst.sync_info
                if si and si.on_wait and len(si.on_wait) > 1:
                    waits = list(si.on_wait)
                    for w in waits[:-1]:
                        nop = bass_rust.InstNoOp(
                            name=f"I-waitsplit-{nc.next_id()}", ins=[], outs=[])
                        nop.engine = inst.engine
                        nop.sync_info = mybir.SyncInfo(on_wait=[w], on_update=[])
                        nc.register_instruction(nop, overwrite=True)
                        out.append(nop)
                        n += 1
                    si.on_wait = [waits[-1]]
                out.append(inst)
            bb.instructions = out
    return n


def build_program(has_bias=False, cpb=CPB):
    _apply_tile_patch()
    import concourse.bass as bass
    import concourse.mybir as mybir
    from concourse.tile import TileContext

    dt = mybir.dt
    f32 = dt.float32
    f16 = dt.float16
    Alu = mybir.AluOpType
    Act = mybir.ActivationFunctionType
    AX = mybir.AxisListType.X

    nc = bass.Bass()
    offs16, tot16 = _offsets(layout16(cpb))
    offs32, tot32 = _offsets(LAYOUT32)
    b16_d = nc.declare_dram_parameter("b16", [tot16], f16, isOutput=False)
    b32_d = nc.declare_dram_parameter("b32", [tot32], f32, isOutput=False)
    y_d = nc.declare_dram_parameter("y", [1, G], f32, isOutput=True)

    def part16(name):
        o, n = offs16[name]
        return b16_d[o:o + n]

    def part32(name):
        o, n = offs32[name]
        return b32_d[o:o + n]

    # DRAM scratch for broadcast bounces
    t3_dr = [nc.dram_tensor(f"t3d{i}", [sz, N], f16) for i, (s, sz) in enumerate(BATCHES)]
    inv_dr = nc.dram_tensor("invd", [1, 1], f32)

    with TileContext(nc) as tc:
        with contextlib.ExitStack() as stack:
            ep = lambda *a, **k: stack.enter_context(tc.tile_pool(*a, **k))
            cpool = ep(name="const", bufs=1)
            atpool = ep(name="at", bufs=4)
            xpool = ep(name="xin", bufs=3)
            ohpool = ep(name="oh", bufs=6)
            h1pool = ep(name="h1", bufs=5)
            h2pool = ep(name="h2", bufs=5)
            h3pool = ep(name="h3", bufs=16)
            hnpool = ep(name="hn", bufs=4)
            upool = ep(name="usb", bufs=3)
            spool = ep(name="stats", bufs=1)
            tkpool = ep(name="tkb", bufs=2)
            scpool = ep(name="scratch", bufs=4)
            psA = ep(name="psA", bufs=2, space="PSUM")
            psU = ep(name="psU", bufs=2, space="PSUM")
            psH = ep(name="psH", bufs=2, space="PSUM")
            psB = ep(name="psB", bufs=1, space="PSUM")
            psS = ep(name="psS", bufs=1, space="PSUM")
            # ---- prefetch first graphs' inputs before heavy consts ------
            gdata = {}

            def load_graph(g):
                sd = xpool.tile([128, 8 * cpb], f16, tag="sd", name=f"sd{g}")
                gsz = 128 * 8 * cpb
                nc.scalar.dma_start(
                    out=sd, in_=part16("sd")[g * gsz:(g + 1) * gsz]
                    .rearrange("(p c) -> p c", p=128))
                xs = xpool.tile([128, 4, 4], f16, tag="xs", name=f"xs{g}")
                nc.sync.dma_start(
                    out=xs, in_=part16("xs")[g * N * 4:(g + 1) * N * 4]
                    .rearrange("(a p f) -> p a f", a=4, p=128, f=4))
                xt = xpool.tile([4, N], f16, tag="xt", name=f"xt{g}")
                nc.sync.dma_start(
                    out=xt, in_=part16("xt")[g * 4 * N:(g + 1) * 4 * N]
                    .rearrange("(a n) -> a n", a=4))
                gdata[g] = (sd, xs, xt)

            ws1_sb = cpool.tile([4, D], f16)
            nc.sync.dma_start(out=ws1_sb, in_=part16("ws1").rearrange("(a m) -> a m", a=4))
            wr1_sb = cpool.tile([4, D], f16)
            nc.sync.dma_start(out=wr1_sb, in_=part16("wr1").rearrange("(a m) -> a m", a=4))
            iota_bc = cpool.tile([128, N], f16)
            nc.sync.dma_start(out=iota_bc, in_=part16("iota")
                              .rearrange("(o n) -> o n", o=1).broadcast_to([128, N]))
            for _pg in range(3):
                load_graph(_pg)

            # ---- one-time constants -------------------------------------
            wst2_sb = cpool.tile([128, 4, D], f16)
            nc.sync.dma_start(out=wst2_sb, in_=part16("wst2")
                              .rearrange("(a p m) -> p a m", a=4, p=128, m=D))
            wst3_sb = cpool.tile([128, 4, D], f16)
            nc.sync.dma_start(out=wst3_sb, in_=part16("wst3")
                              .rearrange("(a p m) -> p a m", a=4, p=128, m=D))
            b_sb = []
            for bn in ("b1", "b2", "b3"):
                t = cpool.tile([128, 2, 1], f32, name=f"b_sb_{bn}")
                nc.sync.dma_start(out=t, in_=part32(bn)
                                  .rearrange("(a p o) -> p a o", a=2, p=128, o=1))
                b_sb.append(t)
            if has_bias:
                b1r_sb = cpool.tile([1, D], f16)
                nc.sync.dma_start(out=b1r_sb, in_=part16("b1r").rearrange("(o m) -> o m", o=1))
                b2r_sb = cpool.tile([1, D], f16)
                nc.sync.dma_start(out=b2r_sb, in_=part16("b2r").rearrange("(o m) -> o m", o=1))
                ones1 = cpool.tile([1, N], f16)
                nc.vector.memset(ones1, 1.0)
            pmat_sb = cpool.tile([128, 2, G, TBMAX], f16)
            nc.sync.dma_start(out=pmat_sb, in_=part16("pmat")
                              .rearrange("(p f g t) -> p f g t", p=128, f=2, g=G, t=TBMAX))
            prow_sb = cpool.tile([1, D], f32)
            nc.sync.dma_start(out=prow_sb, in_=part32("prow").rearrange("(o m) -> o m", o=1))
            mw1_sb = cpool.tile([128, 4, D], f16)
            nc.sync.dma_start(out=mw1_sb, in_=part16("mw1")
                              .rearrange("(a p m) -> p a m", a=4, p=128, m=D))
            mw2_sb = cpool.tile([128, 2, D // 2], f16)
            nc.sync.dma_start(out=mw2_sb, in_=part16("mw2")
                              .rearrange("(a p m) -> p a m", a=2, p=128, m=D // 2))
            mw3_sb = cpool.tile([128, 1], f16)
            nc.sync.dma_start(out=mw3_sb, in_=part16("mw3").rearrange("(p o) -> p o", p=128))
            c1_sb = cpool.tile([128, 2, 1], f32)
            nc.sync.dma_start(out=c1_sb, in_=part32("c1")
                              .rearrange("(a p o) -> p a o", a=2, p=128, o=1))
            c2_sb = cpool.tile([128, 1], f32)
            nc.sync.dma_start(out=c2_sb, in_=part32("c2").rearrange("(p o) -> p o", p=128))
            c3_sb = cpool.tile([1, 1], f32)
            nc.sync.dma_start(out=c3_sb, in_=part32("c3").rearrange("(o q) -> o q", o=1))

            # inv = 1 / ||p||  (the reference's +1e-16 is numerically inert)
            pnorm2 = spool.tile([1, 1], f32)
            sq_scratch = spool.tile([1, D], f32)
            nc.scalar.activation(sq_scratch, prow_sb, Act.Square, accum_out=pnorm2)
            pnorm = spool.tile([1, 1], f32)
            nc.scalar.activation(pnorm, pnorm2, Act.Sqrt, bias=0.0, scale=1.0)
            inv1 = spool.tile([1, 1], f32)
            nc.vector.reciprocal(inv1, pnorm)
            nc.sync.dma_start(out=inv_dr[:, :], in_=inv1)
            invB = spool.tile([TBMAX, 1], f32)
            nc.sync.dma_start(out=invB, in_=inv_dr.broadcast_to([TBMAX, 1]))

            # ---- per-graph accumulators ---------------------------------
            zmax12 = [spool.tile([128, G, 2], f32, tag=f"zmax12_{l}", name=f"zmax12_{l}")
                      for l in range(2)]
            zmax3 = [spool.tile([128, G], f32, tag=f"zmax3{fh}", name=f"zmax3{fh}")
                     for fh in range(2)]
            zsum = [[spool.tile([128, G], f32, tag=f"zsum{l}{fh}", name=f"zsum{l}{fh}")
                     for fh in range(2)] for l in range(3)]

            h3_keep = []
            sc_ps = None

            batch_of = {}
            for bi, (s0, sz) in enumerate(BATCHES):
                for j in range(sz):
                    batch_of[s0 + j] = (bi, j, s0, sz)

            for g in range(G):
                bi, j, bstart, bsize = batch_of[g]

                # ---- input DMAs (3 graphs prefetched ahead) -------------
                if g not in gdata:
                    load_graph(g)
                if g + 3 < G and (g + 3) not in gdata:
                    load_graph(g + 3)
                sd_sb, xs_sb, xt_sb = gdata.pop(g)

                # ---- build AT[s, d] = #edges s->d from the edge list ----
                # Edges are host-sorted by src block b = s>>7 and padded to
                # cpb 128-edge chunks per block (pad: s_in_block=0, d=-1 so
                # the dst one-hot row is all-zero). sd col layout: col
                # (o*4 + b)*cpb + ch with o=0 src-in-block ids, o=1 dst ids.
                # One-hot rows via iota compare; counts via PE accumulation;
                # the src block is known per chunk, so one matmul per chunk.
                sdf = ohpool.tile([128, 8 * cpb], f32, tag="sdf", name=f"sdf{g}")
                nc.scalar.copy(sdf, sd_sb)
                at_sb = atpool.tile([128, 4, N], f16, tag="at", name=f"at{g}")
                for b in range(4):
                    at_ps = psA.tile([128, N], f32, tag="psa", name=f"atps{g}_{b}")
                    for ch in range(cpb):
                        sc = b * cpb + ch
                        dc = (4 + b) * cpb + ch
                        soh = ohpool.tile([128, 128], f16, tag="soh",
                                          name=f"soh{g}_{b}_{ch}")
                        nc.vector.tensor_scalar(soh, iota_bc[:, 0:128],
                                                sdf[:, sc:sc + 1], None, Alu.is_equal)
                        doh = ohpool.tile([128, N], f16, tag="doh",
                                          name=f"doh{g}_{b}_{ch}")
                        nc.vector.tensor_scalar(doh, iota_bc, sdf[:, dc:dc + 1],
                                                None, Alu.is_equal)
                        nc.tensor.matmul(at_ps, soh, doh,
                                         start=(ch == 0), stop=(ch == cpb - 1))
                    nc.scalar.copy(at_sb[:, b, :], at_ps)

                # ---- layer 1 -------------------------------------------
                u1_ps = psB.tile([4, N], f32, tag="psb", name=f"u1ps{g}")
                for nb in range(4):
                    nc.tensor.matmul(u1_ps, xs_sb[:, nb, :], at_sb[:, nb, :],
                                     start=(nb == 0), stop=(nb == 3))
                u1_sb = xpool.tile([4, N], f16, tag="u1")
                nc.scalar.copy(u1_sb, u1_ps)

                # (b) normal h1 [node, feat] for the L2 A-step stationary
                h1n = hnpool.tile([128, 4, D], f16, tag="hn")
                for hh in range(2):
                    hb_ps = psB.tile([128, 2, D], f32, tag="psb", name=f"hb1_{g}_{hh}")
                    for nbi in range(2):
                        nb = hh * 2 + nbi
                        sl = slice(nb * 128, (nb + 1) * 128)
                        nc.tensor.matmul(hb_ps[:, nbi, :], xt_sb[:, sl], ws1_sb,
                                         start=True, stop=False)
                        nc.tensor.matmul(hb_ps[:, nbi, :], u1_sb[:, sl], wr1_sb,
                                         start=False, stop=(not has_bias))
                        if has_bias:
                            nc.tensor.matmul(hb_ps[:, nbi, :], ones1[:, sl], b1r_sb,
                                             start=False, stop=True)
                    nc.scalar.activation(h1n[:, hh * 2:(hh + 1) * 2, :], hb_ps,
                                         Act.Relu, bias=0.0, scale=1.0)

                # (a) T-land H1
                H1 = h1pool.tile([128, 2, N], f16, tag="H1")
                for mh in range(2):
                    h_ps = psH.tile([128, N], f32, tag="psh")
                    nc.tensor.matmul(h_ps, ws1_sb[:, mh * 128:(mh + 1) * 128], xt_sb,
                                     start=True, stop=False)
                    nc.tensor.matmul(h_ps, wr1_sb[:, mh * 128:(mh + 1) * 128], u1_sb,
                                     start=False, stop=True)
                    nc.scalar.activation(H1[:, mh, :], h_ps, Act.Relu,
                                         bias=b_sb[0][:, mh, :], scale=1.0,
                                         accum_out=zsum[0][mh][:, g:g + 1])
                nc.vector.reduce_max(zmax12[0][:, g, :], H1, axis=AX)

                # ---- layer 2 -------------------------------------------
                u_sb = upool.tile([128, 2, N], f16, tag="u")
                for fh in range(2):
                    u_ps = psU.tile([128, N], f32, tag="psu", name=f"u_ps2_{g}_{fh}")
                    for nb in range(4):
                        nc.tensor.matmul(u_ps, h1n[:, nb, fh * 128:(fh + 1) * 128],
                                         at_sb[:, nb, :], start=(nb == 0), stop=(nb == 3))
                    nc.scalar.copy(u_sb[:, fh, :], u_ps)

                h2n = hnpool.tile([128, 4, D], f16, tag="hn")
                for hh in range(2):
                    hb_ps2 = psB.tile([128, 2, D], f32, tag="psb", name=f"hb2_{g}_{hh}")
                    for nbi in range(2):
                        nb = hh * 2 + nbi
                        sl = slice(nb * 128, (nb + 1) * 128)
                        for kb in range(4):
                            stat = H1[:, kb, sl] if kb < 2 else u_sb[:, kb - 2, sl]
                            nc.tensor.matmul(hb_ps2[:, nbi, :], stat, wst2_sb[:, kb, :],
                                             start=(kb == 0),
                                             stop=(kb == 3 and not has_bias))
                        if has_bias:
                            nc.tensor.matmul(hb_ps2[:, nbi, :], ones1[:, sl], b2r_sb,
                                             start=False, stop=True)
                    nc.scalar.activation(h2n[:, hh * 2:(hh + 1) * 2, :], hb_ps2,
                                         Act.Relu, bias=0.0, scale=1.0)

                H2 = h2pool.tile([128, 2, N], f16, tag="H2")
                for mh in range(2):
                    h_ps = psH.tile([128, N], f32, tag="psh")
                    for kb in range(4):
                        mov = H1[:, kb, :] if kb < 2 else u_sb[:, kb - 2, :]
                        nc.tensor.matmul(h_ps, wst2_sb[:, kb, mh * 128:(mh + 1) * 128],
                                         mov, start=(kb == 0), stop=(kb == 3))
                    nc.scalar.activation(H2[:, mh, :], h_ps, Act.Relu,
                                         bias=b_sb[1][:, mh, :], scale=1.0,
                                         accum_out=zsum[1][mh][:, g:g + 1])
                nc.vector.reduce_max(zmax12[1][:, g, :], H2, axis=AX)

                # ---- layer 3 (T-land only) ------------------------------
                u_sb3 = upool.tile([128, 2, N], f16, tag="u")
                for fh in range(2):
                    u_ps3 = psU.tile([128, N], f32, tag="psu", name=f"u_ps3_{g}_{fh}")
                    for nb in range(4):
                        nc.tensor.matmul(u_ps3, h2n[:, nb, fh * 128:(fh + 1) * 128],
                                         at_sb[:, nb, :], start=(nb == 0), stop=(nb == 3))
                    nc.scalar.copy(u_sb3[:, fh, :], u_ps3)

                H3 = h3pool.tile([128, 2, N], f16, tag="H3")
                h3_keep.append(H3)
                for mh in range(2):
                    h_ps = psH.tile([128, N], f32, tag="psh")
                    for kb in range(4):
                        mov = H2[:, kb, :] if kb < 2 else u_sb3[:, kb - 2, :]
                        nc.tensor.matmul(h_ps, wst3_sb[:, kb, mh * 128:(mh + 1) * 128],
                                         mov, start=(kb == 0), stop=(kb == 3))
                    nc.scalar.activation(H3[:, mh, :], h_ps, Act.Relu,
                                         bias=b_sb[2][:, mh, :], scale=1.0)

                # ---- score: accumulate row j of the batch tile ----------
                # stationary column j = p_attn, other columns zero, so graph
                # g's scores land in psum row j while other rows add zero.
                if j == 0:
                    sc_ps = psS.tile([bsize, N], f32, tag="psmall", name=f"sc_ps{bi}")
                for fh in range(2):
                    nc.tensor.matmul(sc_ps, pmat_sb[:, fh, g, 0:bsize], H3[:, fh, :],
                                     start=(j == 0 and fh == 0),
                                     stop=(j == bsize - 1 and fh == 1))

                # ---- topk + x3 pooling per batch ------------------------
                if j == bsize - 1:
                    scores = tkpool.tile([TBMAX, N], f32, tag="scores", name=f"scores{bi}")[0:bsize]
                    nc.scalar.copy(scores, sc_ps)
                    tneg = [tkpool.tile([TBMAX, N], f32, tag=f"tneg{i}", name=f"tneg{i}_{bi}")[0:bsize]
                            for i in range(2)]
                    m8 = tkpool.tile([TBMAX, 8], f32, tag="m8", name=f"m8_{bi}")[0:bsize]
                    vthr = tkpool.tile([TBMAX, 1], f32, tag="vthr", name=f"vthr{bi}")[0:bsize]
                    t3 = tkpool.tile([TBMAX, N], f16, tag="t3", name=f"t3_{bi}")[0:bsize]

                    nc.vector.tensor_scalar_mul(tneg[0], scores, -1.0)
                    cur = 0
                    for r in range(12):
                        nc.vector.max(m8, tneg[cur])
                        nc.vector.match_replace(tneg[1 - cur], m8, tneg[cur], -1e30)
                        cur = 1 - cur
                    nc.vector.max(m8, tneg[cur])
                    # threshold = 103rd smallest score = -(m8 col 6)
                    nc.vector.tensor_scalar_mul(vthr, m8[:, 6:7], -1.0)
                    # w = tanh(s/||p||) * mask
                    tt_s = tkpool.tile([TBMAX, N], f32, tag="tt_s", name=f"tt_s{bi}")[0:bsize]
                    nc.scalar.activation(tt_s, scores, Act.Tanh,
                                         bias=0.0, scale=invB[0:bsize])
                    mask_s = tkpool.tile([TBMAX, N], f16, tag="mask_s", name=f"mask_s{bi}")[0:bsize]
                    nc.vector.tensor_scalar(mask_s, scores, vthr, None, Alu.is_ge)
                    nc.vector.tensor_tensor(out=t3, in0=tt_s, in1=mask_s, op=Alu.mult)
                    nc.sync.dma_start(out=t3_dr[bi][:, :], in_=t3)

                    bcs, p1s = [], []
                    for jj in range(bsize):
                        bc = scpool.tile([128, N], f16, tag="bc", bufs=14,
                                         name=f"bc{bi}_{jj}")
                        nc.sync.dma_start(
                            out=bc,
                            in_=t3_dr[bi][jj:jj + 1].broadcast_to([128, N]))
                        bcs.append(bc)
                    # pass 1 (DVE): P = H3*w with fused masked-sum accumulation
                    for jj in range(bsize):
                        gg = bstart + jj
                        H3g = h3_keep[gg]
                        for fh in range(2):
                            p1 = scpool.tile([128, N], f16, tag="p1", bufs=8,
                                             name=f"p1_{bi}_{jj}_{fh}")
                            nc.vector.scalar_tensor_tensor(
                                out=p1, in0=H3g[:, fh, :], scalar=1.0,
                                in1=bcs[jj], op0=Alu.mult, op1=Alu.mult,
                                accum_out=zsum[2][fh][:, gg:gg + 1])
                            p1s.append(p1)
                    # pass 2 (DVE): masked max = max(P) (h3>=0; see header comment)
                    for jj in range(bsize):
                        gg = bstart + jj
                        for fh in range(2):
                            nc.vector.reduce_max(zmax3[fh][:, gg:gg + 1],
                                                 p1s[2 * jj + fh], axis=AX)

            # ---- assemble z and run the MLP head ------------------------
            zmx = [scpool.tile([128, G], f16, tag=f"zmx{fh}", name=f"zmx{fh}")
                   for fh in range(2)]
            zmn = [scpool.tile([128, G], f16, tag=f"zmn{fh}", name=f"zmn{fh}")
                   for fh in range(2)]
            for fh in range(2):
                ztmp = scpool.tile([128, G], f32, tag="ztmp")
                nc.vector.tensor_add(ztmp, zmax12[0][:, :, fh], zmax12[1][:, :, fh])
                nc.vector.tensor_add(zmx[fh], ztmp, zmax3[fh])
                ztmp2 = scpool.tile([128, G], f32, tag="ztmp2")
                nc.vector.tensor_add(ztmp2, zsum[0][fh], zsum[1][fh])
                nc.vector.tensor_scalar_mul(ztmp2, ztmp2, 1.0 / N)
                nc.vector.scalar_tensor_tensor(
                    out=zmn[fh], in0=zsum[2][fh], scalar=1.0 / K, in1=ztmp2,
                    op0=Alu.mult, op1=Alu.add)

            z_tiles = [zmx[0], zmx[1], zmn[0], zmn[1]]
            zz1 = scpool.tile([128, 2, G], f16, tag="zz1")
            for mh in range(2):
                mlp_ps = psH.tile([128, G], f32, tag="psh")
                for kb in range(4):
                    nc.tensor.matmul(mlp_ps, mw1_sb[:, kb, mh * 128:(mh + 1) * 128],
                                     z_tiles[kb], start=(kb == 0), stop=(kb == 3))
                nc.scalar.activation(zz1[:, mh, :], mlp_ps, Act.Relu,
                                     bias=c1_sb[:, mh, :], scale=1.0)
            zz2 = scpool.tile([128, G], f16, tag="zz2")
            mlp_ps2 = psH.tile([128, G], f32, tag="psh")
            for kb in range(2):
                nc.tensor.matmul(mlp_ps2, mw2_sb[:, kb, :], zz1[:, kb, :],
                                 start=(kb == 0), stop=(kb == 1))
            nc.scalar.activation(zz2, mlp_ps2, Act.Relu, bias=c2_sb, scale=1.0)
            y_ps = psS.tile([1, G], f32, tag="psmall")
            nc.tensor.matmul(y_ps, mw3_sb, zz2, start=True, stop=True)
            y_sb = scpool.tile([1, G], f32, tag="ysb")
            nc.scalar.activation(y_sb, y_ps, Act.Sigmoid, bias=c3_sb, scale=1.0)
            nc.sync.dma_start(out=y_d[:, :], in_=y_sb)

    _split_multi_waits(nc)
    return nc


# ---------------------------------------------------------------------------
# Host-side packing
# ---------------------------------------------------------------------------

def pack_inputs(inputs):
    x = np.asarray(inputs["x"], np.float32)
    src = np.asarray(inputs["src"]).astype(np.int32)
    dst = np.asarray(inputs["dst"]).astype(np.int32)

    def hf(a):
        return np.ascontiguousarray(np.asarray(a, np.float32).astype(F16))

    ws1 = hf(inputs["Ws1"]); wr1 = hf(inputs["Wr1"])
    wst2 = hf(np.concatenate([inputs["Ws2"], inputs["Wr2"]], axis=0))
    wst3 = hf(np.concatenate([inputs["Ws3"], inputs["Wr3"]], axis=0))
    b1 = np.asarray(inputs["b1"], np.float32).reshape(D, 1)
    b2 = np.asarray(inputs["b2"], np.float32).reshape(D, 1)
    b3 = np.asarray(inputs["b3"], np.float32).reshape(D, 1)
    p = np.asarray(inputs["p_attn"], np.float32)
    pmat = np.zeros((128, 2, G, TBMAX), np.float32)
    for bstart, bsize in BATCHES:
        for j in range(bsize):
            pmat[:, 0, bstart + j, j] = p[:128]
            pmat[:, 1, bstart + j, j] = p[128:]
    prow = np.ascontiguousarray(p.reshape(1, D))
    mw1 = hf(inputs["W1"]); mw2 = hf(inputs["W2"]); mw3 = hf(inputs["W3"])
    c1 = np.asarray(inputs["c1"], np.float32).reshape(D, 1)
    c2 = np.asarray(inputs["c2"], np.float32).reshape(D // 2, 1)
    c3 = np.asarray(inputs["c3"], np.float32).reshape(1, 1)

    shared = dict(ws1=ws1, wr1=wr1, wst2=wst2, wst3=wst3, b1=b1, b2=b2, b3=b3,
                  b1r=hf(b1.reshape(1, D)), b2r=hf(b2.reshape(1, D)),
                  pmat=hf(pmat), prow=prow, mw1=mw1, mw2=mw2, mw3=mw3,
                  c1=c1, c2=c2, c3=c3)

    # per-graph local edge ids, host-sorted by src block (s>>7) and padded
    # to CPB 128-edge chunks per block, packed as [128 edge-in-chunk, 8*CPB]
    # f16: col (o*4+b)*CPB+ch holds chunk ch of block b (o=0: src id within
    # block, o=1: dst id; pad rows are s=0/d=-1 so the dst one-hot row is
    # all-zero). Values < 512 are exact in f16; the device one-hots them via
    # iota compare and builds the adjacency-count matrix on the PE.
    B = NCORES * G
    cpb = CPB
    offs = (np.arange(B, dtype=np.int32) * N)[:, None]
    s_all = src.reshape(B, E) - offs
    d_all = dst.reshape(B, E) - offs
    blk = s_all >> 7
    order = np.argsort(blk, axis=1, kind="stable")
    blk_s = np.take_along_axis(blk, order, axis=1)
    s_sorted = np.take_along_axis(s_all, order, axis=1) - blk_s * 128
    d_sorted = np.take_along_axis(d_all, order, axis=1)
    cnts = np.stack([(blk == b).sum(axis=1) for b in range(4)], axis=1)
    need = int(np.ceil(cnts.max() / 128))
    if need > cpb:  # never taken for the reference distribution; keeps
        cpb = need  # kernel() correct for arbitrary edge lists
    starts = np.concatenate(
        [np.zeros((B, 1), np.int64), np.cumsum(cnts, axis=1)[:, :3]], axis=1)
    rank = np.arange(E, dtype=np.int64)[None, :] - np.take_along_axis(
        starts, blk_s.astype(np.int64), axis=1)
    dest = (np.arange(B, dtype=np.int64)[:, None] * 4 + blk_s) * (cpb * 128) + rank
    sp = np.zeros((B * 4 * cpb * 128,), np.int32)
    dpad = np.full((B * 4 * cpb * 128,), -1, np.int32)
    sp[dest.reshape(-1)] = s_sorted.reshape(-1)
    dpad[dest.reshape(-1)] = d_sorted.reshape(-1)
    # [B, 4, cpb, 128] -> [B, 128, 4, cpb] (partition = edge-in-chunk)
    sp4 = sp.reshape(B, 4, cpb, 128).transpose(0, 3, 1, 2)
    dp4 = dpad.reshape(B, 4, cpb, 128).transpose(0, 3, 1, 2)
    sd_all = np.empty((B, 128, 2, 4, cpb), F16)
    sd_all[:, :, 0] = sp4
    sd_all[:, :, 1] = dp4
    sd_all = sd_all.reshape(B, 128, 8 * cpb)
    xb = x.reshape(B, N, 4)
    xs_all = xb.astype(F16)
    xt_all = np.ascontiguousarray(xb.transpose(0, 2, 1)).astype(F16)
    iota_row = np.arange(N, dtype=np.float32).astype(F16)

    # assemble the two flat blobs (layout16/LAYOUT32 order)
    shared16 = np.concatenate([
        shared["ws1"].ravel(), shared["wr1"].ravel(),
        shared["wst2"].ravel(), shared["wst3"].ravel(),
        shared["b1r"].ravel(), shared["b2r"].ravel(),
        shared["pmat"].ravel(),
        shared["mw1"].ravel(), shared["mw2"].ravel(), shared["mw3"].ravel(),
    ]).astype(F16)
    b32 = np.concatenate([
        shared["prow"].ravel(),
        shared["b1"].ravel(), shared["b2"].ravel(), shared["b3"].ravel(),
        shared["c1"].ravel(), shared["c2"].ravel(), shared["c3"].ravel(),
    ]).astype(np.float32)

    in_maps = []
    for c in range(NCORES):
        g0 = c * G
        b16 = np.concatenate([
            sd_all[g0:g0 + G].ravel(), iota_row, xs_all[g0:g0 + G].ravel(),
            xt_all[g0:g0 + G].ravel(), shared16,
        ])
        in_maps.append(dict(b16=b16, b32=b32, _cpb=cpb))
    return in_maps


def has_nonzero_bias(inputs):
    return any(np.any(np.asarray(inputs[k]) != 0) for k in ("b1", "b2"))


def kernel(**inputs):
    from concourse.bass_utils import run_bass_kernel_spmd

    in_maps = pack_inputs(inputs)
    cpb = in_maps[0].pop("_cpb")
    for m in in_maps[1:]:
        m.pop("_cpb")
    nc = build_program(has_bias=has_nonzero_bias(inputs), cpb=cpb)
    try:
        res = run_bass_kernel_spmd(nc, in_maps, list(range(NCORES)))
    except Exception:
        # one retry: the axon tunnel / NRT occasionally reports a transient
        # unrecoverable-exec-unit on the first execute after another process
        # tore down; a fresh attempt succeeds.
        res = run_bass_kernel_spmd(nc, in_maps, list(range(NCORES)))
    y = np.concatenate([np.asarray(res.results[c]["y"], np.float32).reshape(-1)
                        for c in range(NCORES)])
    return y

